# revision 23
# baseline (speedup 1.0000x reference)
"""Euclidean contrastive loss on 8 Trainium2 NeuronCores (Bass/Tile).

Triangle-band scheme (SPMD, one program for all 8 cores; per-core data
rotated so all device offsets are compile-time constants):
  - Rotation: core c's local row r = global row (c*1024 + r) mod 8192 and
    local col j = global col (c*1024 + j) mod 8192, so circular block
    distance d = (colblock - rowblock) mod 64 is layout-invariant.
  - Each local row block m (128 rows) computes cols [128m, 128m+4224):
    its diag block (d=0) plus a forward band d=1..32.  Globally every
    unordered block pair at d=1..31 is computed once, d=32 twice, d=0 once.
  - Device per core:
      * 16 linear DMAs load host-pretransposed tokT chunks (bf16).
      * norms: sq = chunk^2 (DVE), ones-matmul col-sums over k (PE),
        Abs_reciprocal_sqrt (ACT) -> inv_bc bf16 (bcast over partitions).
      * normalize+cast to fp8 DoubleRow layout t8[128, KT, N]
        (group 0 on DVE for latency, groups 1-3 on idle GPSIMD).
      * sim: fp8 DoubleRow matmuls (2 k-subtiles/instr) -> PSUM;
        diag fix (sim_ii -= 2); dist = Sqrt(A - A*sim) fp16 (ACT).
      * numerator: pos = (lab_bc == lab_row) (DVE), prod = dist*pos,
        row-accumulate full/diag/d32 partials (DVE).
      * exp(-dist)*2^38 in place over dist (ACT) with row-sum accum
        (the 2^38 scale keeps values inside fp16 range for the colsum);
        colsum-of-exp over d=1..31 via ones-matmuls accumulated per
        absolute 512-chunk across the phase's blocks (PE) -> colacc.
  - Host: rowsum = own + mirrored colacc, LSE = ln(rowsum) - 38 ln2,
    NUM = sum(2*numpart - sdiag - s32) - N*(2/tau) + sum(npos*LSE),
    loss = NUM / sum(npos).
"""

import os
import sys

import numpy as np
import ml_dtypes

try:
    import concourse.bass as bass  # noqa: F401
except ImportError:  # harness runs from a bare directory
    for p in ("/opt/trn_rl_repo", os.path.expanduser("~/.axon_site/_ro/trn_rl_repo")):
        if os.path.isdir(p) and p not in sys.path:
            sys.path.insert(0, p)
    import concourse.bass as bass  # noqa: F401

import concourse.mybir as mybir
import concourse.tile as tile
from concourse import bacc, bass_utils
from concourse.tile import add_dep_helper

N, D, NCORES = 8192, 512, 8
RPC = N // NCORES        # 1024 rows per core
NB = RPC // 128          # 8 row blocks of 128
KT = D // 128            # 4 contraction tiles
GW = 2048                # column group width (prep granularity)
NG = N // GW             # 4 column groups
CH = 1024                # sim psum chunk width
BAND = 4096              # forward band width (d=1..32)
REG = 128 + BAND         # computed region width per block
CSW = BAND - 128         # colsum width per block (d=1..31)
PH = 2
BPP = NB // PH
NCLS = 100
NQ = 10                  # colacc 512-chunks (cols [0, 5120))
W = 5120                 # max local column any block touches (128*7+4224)
GW2 = 1024               # prep group width
NG2 = W // GW2           # 5 prep groups
ESC = 38.0               # exp values scaled by 2^38 to survive fp16

BF16 = mybir.dt.bfloat16
FP16 = mybir.dt.float16
FP32 = mybir.dt.float32
FP8 = mybir.dt.float8e4
OP = mybir.AluOpType
AF = mybir.ActivationFunctionType
DR = mybir.MatmulPerfMode.DoubleRow

_CACHE: dict = {}
last_results = None  # test harness reads exec_time_ns from here


def _build(tau: float):
    nc = bacc.Bacc(
        "TRN2",
        target_bir_lowering=False,
        debug=False,
        enable_asserts=False,
        num_devices=NCORES,
    )
    tokT = nc.dram_tensor("tokT", [D, N], BF16, kind="ExternalInput")
    lab_bc = nc.dram_tensor("lab_bc", [128, N], BF16, kind="ExternalInput")
    lab_rows = nc.dram_tensor("lab_rows", [128, NB], FP32, kind="ExternalInput")
    out_p = nc.dram_tensor("part", [128, 4 * NB], FP32, kind="ExternalOutput")
    out_cs = nc.dram_tensor("cs", [1, NQ * 512], BF16, kind="ExternalOutput")

    A = 2.0 / (tau * tau)  # (dist/tau)^2 = A - A*sim

    act_chain = []  # ACT instructions in required execution order

    def act(*args, **kwargs):
        inst = nc.scalar.activation(*args, **kwargs)
        act_chain.append(inst)
        return inst

    with tile.TileContext(nc) as tc:
        with (
            tc.tile_pool(name="persist", bufs=1) as pp,
            tc.tile_pool(name="traw", bufs=8) as trp,
            tc.tile_pool(name="dist", bufs=6) as dsp,
            tc.tile_pool(name="sq", bufs=4) as sqp,
            tc.tile_pool(name="msk", bufs=5) as mkp,
            tc.tile_pool(name="jk", bufs=1) as jkp,
            tc.tile_pool(name="psim", bufs=3, space="PSUM") as psim,
            tc.tile_pool(name="psm", bufs=2, space="PSUM") as psm,
        ):
            # ---- persistent tiles ----
            t8 = pp.tile([128, KT, W], FP8, tag="t8")
            Lc = pp.tile([128, W], BF16, tag="Lc")
            lr = pp.tile([128, NB], FP32, tag="lr")
            inv_bc = pp.tile([128, W], BF16, tag="inv_bc")
            dm0 = pp.tile([128, 128], BF16, tag="dm0")
            ones = pp.tile([128, 128], BF16, tag="ones")
            colacc = pp.tile([128, NQ * 512], BF16, tag="colacc")
            # partials: [rowsum | numpart | sdiag | s32]
            parts = pp.tile([128, 4 * NB], FP32, tag="parts")
            biasA = pp.tile([128, 1], FP32, tag="biasA")
            biasE = pp.tile([128, 1], FP32, tag="biasE")

            nc.gpsimd.memset(ones[:], 1.0)
            nc.gpsimd.memset(biasA[:], float(A))
            nc.gpsimd.memset(biasE[:], float(ESC * np.log(2.0)))
            nc.gpsimd.memset(colacc[:], 0.0)

            # ---- diag mask dm0[p, f] = (f == p) ----
            iot = mkp.tile([128, 128], mybir.dt.int32, tag="iot")
            nc.gpsimd.iota(iot[:], pattern=[[1, 128]], base=0, channel_multiplier=-1)
            iotf = mkp.tile([128, 128], FP32, tag="iotf")
            nc.vector.tensor_copy(iotf[:], iot[:])
            nc.vector.tensor_scalar(dm0[:], iotf[:], 0.0, None, op0=OP.is_equal)

            # ---- prep helper: load tokT group, norms, normalize -> fp8 ----
            # only local cols [0, W) are ever used by the band scheme
            def prep_group(g):
                gsl = slice(g * GW2, (g + 1) * GW2)
                tch = []
                for k in range(KT):
                    t = trp.tile([128, GW2], BF16, tag="tr", name=f"tr{g}_{k}")
                    tch.append(t)
                    nc.sync.dma_start(
                        t[:], tokT[k * 128:(k + 1) * 128, gsl],
                    )
                sqt = []
                for k in range(KT):
                    s = sqp.tile([128, GW2], BF16, tag="sq")
                    sqt.append(s)
                    nc.vector.tensor_tensor(s[:], tch[k][:], tch[k][:],
                                            op=OP.mult)
                for c in range(GW2 // 512):
                    csl_g = slice(c * 512, (c + 1) * 512)
                    csl = slice(g * GW2 + c * 512, g * GW2 + (c + 1) * 512)
                    nps = psm.tile([128, 512], FP32, tag="sm", name=f"nps{g}_{c}")
                    for k in range(KT):
                        nc.tensor.matmul(
                            nps[:, :], ones[:, :], sqt[k][:, csl_g],
                            start=(k == 0), stop=(k == KT - 1),
                        )
                    act(inv_bc[:, csl], nps[:], AF.Abs_reciprocal_sqrt)
                # normalize + cast to fp8 DoubleRow layout
                for k in range(KT):
                    nc.vector.tensor_tensor(
                        t8[:, k, gsl], tch[k][:], inv_bc[:, gsl], op=OP.mult,
                    )

            def sim_chunk(dist_m, m, ch):
                base = m * 128
                ps = psim.tile([128, CH], FP32, tag="ps")
                c0 = base + 128 + ch * CH
                for kp in range(KT // 2):
                    lhsT = t8[:, 2 * kp:2 * kp + 2, base:base + 128]
                    for nn in range(CH // 512):
                        cs = slice(c0 + nn * 512, c0 + (nn + 1) * 512)
                        nc.tensor.matmul(
                            ps[:, nn * 512:(nn + 1) * 512],
                            lhsT, t8[:, 2 * kp:2 * kp + 2, cs],
                            start=(kp == 0), stop=(kp == KT // 2 - 1),
                            perf_mode=DR,
                        )
                act(dist_m[:, 128 + ch * CH:128 + (ch + 1) * CH],
                    ps[:], AF.Sqrt, bias=biasA[:], scale=float(-A))

            def sim_diag(dist_m, m):
                base = m * 128
                dps = psm.tile([128, 512], FP32, tag="sm", name=f"dg{m}")
                for kp in range(KT // 2):
                    nc.tensor.matmul(
                        dps[:, 0:128],
                        t8[:, 2 * kp:2 * kp + 2, base:base + 128],
                        t8[:, 2 * kp:2 * kp + 2, base:base + 128],
                        start=(kp == 0), stop=(kp == KT // 2 - 1),
                        perf_mode=DR,
                    )
                nc.vector.scalar_tensor_tensor(
                    out=dps[:, 0:128], in0=dm0[:], scalar=-2.0,
                    in1=dps[:, 0:128], op0=OP.mult, op1=OP.add,
                )
                act(dist_m[:, 0:128], dps[:, 0:128], AF.Sqrt,
                    bias=biasA[:], scale=float(-A))

            def make_pos(m):
                base = m * 128
                pos = mkp.tile([128, REG], BF16, tag="pos", name=f"pos{m}")
                nc.vector.tensor_scalar(
                    pos[:], Lc[:, base:base + REG], lr[:, m:m + 1], None,
                    op0=OP.is_equal,
                )
                return pos

            def numerator(dist_m, m, pos):
                jnk = jkp.tile([128, BAND - 128], FP16, tag="jnk")
                for lo, hi, col in (
                    (128, BAND, NB + m),
                    (0, 128, 2 * NB + m),      # diag
                    (BAND, REG, 3 * NB + m),   # d=32
                ):
                    nc.vector.scalar_tensor_tensor(
                        out=jnk[:, 0:hi - lo], in0=dist_m[:, lo:hi],
                        scalar=1.0, in1=pos[:, lo:hi],
                        op0=OP.mult, op1=OP.mult,
                        accum_out=parts[:, col:col + 1],
                    )

            # ---- main compute, prep interleaved with phase 0 ----
            def colsum(blocks, dist_of):
                for q in range(NQ):
                    q0, q1 = q * 512, (q + 1) * 512
                    pieces = []
                    for m in blocks:
                        lo = max(q0, m * 128 + 128)
                        hi = min(q1, m * 128 + 128 + CSW)
                        if lo < hi:
                            pieces.append((m, lo, hi))
                    if not pieces:
                        continue
                    pieces.sort(key=lambda t: t[1] - t[2])  # widest first
                    cps = psm.tile([128, 512], FP32, tag="sm",
                                   name=f"cs{blocks[0]}_{q}")
                    for i, (m, lo, hi) in enumerate(pieces):
                        nc.tensor.matmul(
                            cps[:, lo - q0:hi - q0],
                            ones[:, :],
                            dist_of[m][:, lo - m * 128:hi - m * 128],
                            start=(i == 0), stop=(i == len(pieces) - 1),
                        )
                    lo = min(p[1] for p in pieces)
                    hi = max(p[2] for p in pieces)
                    nc.vector.tensor_tensor(
                        colacc[:, lo:hi], colacc[:, lo:hi],
                        cps[:, lo - q0:hi - q0], op=OP.add,
                    )

            for g in range(3):
                prep_group(g)
            nc.sync.dma_start(Lc[:], lab_bc[:, 0:W])
            nc.sync.dma_start(lr[:], lab_rows[:, :])
            # phase 0 round A interleaved with remaining prep groups so the
            # ACT chain releases rsqrt(g3)/rsqrt(g4) early
            dist_of = {}
            pos_of = {}
            for m in range(BPP):
                dist_of[m] = dsp.tile([128, REG], FP16, tag="ds",
                                      name=f"dist{m}")
                sim_diag(dist_of[m], m)
                sim_chunk(dist_of[m], m, 0)
            prep_group(3)
            for m in range(BPP):
                pos_of[m] = make_pos(m)
                sim_chunk(dist_of[m], m, 1)
            prep_group(4)
            # phase 0 round B: remaining chunks, numerator, exp, colsum
            for m in range(BPP):
                sim_chunk(dist_of[m], m, 2)
                sim_chunk(dist_of[m], m, 3)
                numerator(dist_of[m], m, pos_of[m])
            for m in range(BPP):
                act(dist_of[m][:], dist_of[m][:], AF.Exp, scale=-1.0,
                    bias=biasE[:], accum_out=parts[:, m:m + 1])
            colsum(list(range(BPP)), dist_of)
            # phase 1
            dist_of = {}
            for m in range(BPP, NB):
                pos_of[m] = make_pos(m)
            for m in range(BPP, NB):
                dist_of[m] = dsp.tile([128, REG], FP16, tag="ds",
                                      name=f"dist{m}")
                sim_diag(dist_of[m], m)
                for ch in range(BAND // CH):
                    sim_chunk(dist_of[m], m, ch)
                numerator(dist_of[m], m, pos_of[m])
            for m in range(BPP, NB):
                act(dist_of[m][:], dist_of[m][:], AF.Exp, scale=-1.0,
                    bias=biasE[:], accum_out=parts[:, m:m + 1])
                if m == NB - 3:
                    colsum([BPP, BPP + 1], dist_of)
            colsum([NB - 2, NB - 1], dist_of)

            nc.sync.dma_start(out_p[:, :], parts[:])
            nc.sync.dma_start(out_cs[:, :], colacc[0:1, :])

            # ---- pin ACT execution order (stop table-set thrash) ----
            for a, b in zip(act_chain, act_chain[1:]):
                add_dep_helper(b.ins, a.ins, reason="act table-set order")

    nc.compile()
    return nc


def _get_program(tau: float):
    if tau not in _CACHE:
        _CACHE[tau] = _build(tau)
    return _CACHE[tau]


def make_in_maps(tokens: np.ndarray, labels: np.ndarray):
    bf = ml_dtypes.bfloat16
    tokT_full = np.ascontiguousarray(
        np.asarray(tokens, dtype=np.float32).astype(bf).T)  # [D, N]
    lab = np.asarray(labels).astype(np.float32)
    in_maps = []
    for c in range(NCORES):
        sh = c * RPC
        tokT_rot = np.ascontiguousarray(np.roll(tokT_full, -sh, axis=1))
        lab_rot = np.roll(lab, -sh)
        lab_bc = np.ascontiguousarray(
            np.broadcast_to(lab_rot.astype(bf)[None, :], (128, N))
        )
        lab_rows = np.ascontiguousarray(
            lab_rot[:RPC].reshape(NB, 128).T.astype(np.float32)
        )
        in_maps.append({
            "tokT": tokT_rot,
            "lab_bc": lab_bc,
            "lab_rows": lab_rows,
        })
    return in_maps


def _install_ntff_hook_shim():
    """Provide antenv.axon_hooks if the image lacks it (NTFF profiling via
    direct ctypes calls into libaxon_pjrt.so)."""
    try:
        from antenv.axon_hooks import get_axon_ntff_profile_hook  # noqa: F401
        return True
    except ImportError:
        pass
    so_path = "/opt/axon/libaxon_pjrt.so"
    if not os.path.exists(so_path):
        return False
    import contextlib
    import ctypes
    import types

    lib = ctypes.CDLL(so_path)
    if not hasattr(lib, "axon_start_nrt_profile"):
        return False
    lib.axon_start_nrt_profile.argtypes = [
        ctypes.POINTER(ctypes.c_int64), ctypes.c_size_t,
    ]
    lib.axon_start_nrt_profile.restype = ctypes.c_int64
    lib.axon_stop_nrt_profile.argtypes = [ctypes.c_char_p]
    lib.axon_stop_nrt_profile.restype = ctypes.c_int64

    @contextlib.contextmanager
    def _hook(output_dir, device_ids):
        import jax
        jax.devices()
        if device_ids:
            ids = (ctypes.c_int64 * len(device_ids))(*device_ids)
            rc = lib.axon_start_nrt_profile(ids, len(device_ids))
        else:
            rc = lib.axon_start_nrt_profile(None, 0)
        if rc != 0:
            raise RuntimeError(f"axon_start_nrt_profile rc={rc}")
        try:
            yield
        finally:
            n = lib.axon_stop_nrt_profile(str(output_dir).encode())
            if n < 0:
                raise RuntimeError(f"axon_stop_nrt_profile rc={n}")
            print(f"profile: {n} file(s) written to {output_dir}")

    mod = types.ModuleType("antenv.axon_hooks")
    mod.get_axon_ntff_profile_hook = lambda: _hook
    mod.set_axon_ntff_profile_hook = lambda h: None
    sys.modules["antenv.axon_hooks"] = mod
    return True


def kernel(tokens, labels, temperature=0.07):
    global last_results
    tau = float(temperature)
    nc = _get_program(tau)
    lab = np.asarray(labels).astype(np.int64)
    in_maps = make_in_maps(tokens, lab)
    trace = bool(int(os.environ.get("KBENCH_TRACE", "0")))
    if trace:
        trace = _install_ntff_hook_shim()
    res = bass_utils.run_bass_kernel_spmd(
        nc, in_maps, core_ids=list(range(NCORES)),
        trace=trace,
    )
    last_results = res

    counts = np.bincount(lab, minlength=NCLS)
    npos = (counts[lab] - 1).astype(np.float64)
    den = npos.sum()

    rowsum = np.zeros(N, dtype=np.float64)
    extra = np.zeros(N, dtype=np.float64)
    num = 0.0
    for c in range(NCORES):
        p = res.results[c]["part"].astype(np.float64)   # [128, 4*NB]
        cs = res.results[c]["cs"].astype(np.float64)    # [1, NQ*512]
        base = c * RPC
        for m in range(NB):
            gl = base + m * 128
            rowsum[gl:gl + 128] = p[:, m]
        num += 2.0 * p[:, NB:2 * NB].sum()      # mid (d=1..31) counted twice
        num += p[:, 2 * NB:3 * NB].sum()        # diag once
        num += p[:, 3 * NB:4 * NB].sum()        # d=32 once
        loc = np.zeros(N, dtype=np.float64)
        loc[:NQ * 512] = cs[0]
        extra += np.roll(loc, base)
    rowsum += extra
    num -= N * (2.0 / tau)                 # self-pair correction
    lse = np.log(rowsum) - ESC * np.log(2.0)
    num += (npos * lse).sum()
    return np.float32(num / den)


# revision 25
# speedup vs baseline: 1.0262x; 1.0262x over previous
"""Euclidean contrastive loss on 8 Trainium2 NeuronCores (Bass/Tile).

Triangle-band scheme (SPMD, one program for all 8 cores; per-core data
rotated so all device offsets are compile-time constants):
  - Rotation: core c's local row r = global row (c*1024 + r) mod 8192 and
    local col j = global col (c*1024 + j) mod 8192, so circular block
    distance d = (colblock - rowblock) mod 64 is layout-invariant.
  - Each local row block m (128 rows) computes cols [128m, 128m+4224):
    its diag block (d=0) plus a forward band d=1..32.  Globally every
    unordered block pair at d=1..31 is computed once, d=32 twice, d=0 once.
  - Device per core:
      * 16 linear DMAs load host-pretransposed tokT chunks (bf16).
      * norms: sq = chunk^2 (DVE), ones-matmul col-sums over k (PE),
        Abs_reciprocal_sqrt (ACT) -> inv_bc bf16 (bcast over partitions).
      * normalize+cast to fp8 DoubleRow layout t8[128, KT, N]
        (group 0 on DVE for latency, groups 1-3 on idle GPSIMD).
      * sim: fp8 DoubleRow matmuls (2 k-subtiles/instr) -> PSUM;
        diag fix (sim_ii -= 2); dist = Sqrt(A - A*sim) fp16 (ACT).
      * numerator: pos = (lab_bc == lab_row) (DVE), prod = dist*pos,
        row-accumulate full/diag/d32 partials (DVE).
      * exp(-dist)*2^38 in place over dist (ACT) with row-sum accum
        (the 2^38 scale keeps values inside fp16 range for the colsum);
        colsum-of-exp over d=1..31 via ones-matmuls accumulated per
        absolute 512-chunk across the phase's blocks (PE) -> colacc.
  - Host: rowsum = own + mirrored colacc, LSE = ln(rowsum) - 38 ln2,
    NUM = sum(2*numpart - sdiag - s32) - N*(2/tau) + sum(npos*LSE),
    loss = NUM / sum(npos).
"""

import os
import sys

import numpy as np
import ml_dtypes

try:
    import concourse.bass as bass  # noqa: F401
except ImportError:  # harness runs from a bare directory
    for p in ("/opt/trn_rl_repo", os.path.expanduser("~/.axon_site/_ro/trn_rl_repo")):
        if os.path.isdir(p) and p not in sys.path:
            sys.path.insert(0, p)
    import concourse.bass as bass  # noqa: F401

import concourse.mybir as mybir
import concourse.tile as tile
from concourse import bacc, bass_utils
from concourse.tile import add_dep_helper

N, D, NCORES = 8192, 512, 8
RPC = N // NCORES        # 1024 rows per core
NB = RPC // 128          # 8 row blocks of 128
KT = D // 128            # 4 contraction tiles
GW = 2048                # column group width (prep granularity)
NG = N // GW             # 4 column groups
CH = 1024                # sim psum chunk width
BAND = 4096              # forward band width (d=1..32)
REG = 128 + BAND         # computed region width per block
CSW = BAND - 128         # colsum width per block (d=1..31)
PH = 2
BPP = NB // PH
NCLS = 100
NQ = 10                  # colacc 512-chunks (cols [0, 5120))
W = 5120                 # max local column any block touches (128*7+4224)
GW2 = 1024               # prep group width
NG2 = W // GW2           # 5 prep groups
ESC = 38.0               # exp values scaled by 2^38 to survive fp16

BF16 = mybir.dt.bfloat16
FP16 = mybir.dt.float16
FP32 = mybir.dt.float32
FP8 = mybir.dt.float8e4
OP = mybir.AluOpType
AF = mybir.ActivationFunctionType
DR = mybir.MatmulPerfMode.DoubleRow

_CACHE: dict = {}
last_results = None  # test harness reads exec_time_ns from here


def _build(tau: float):
    nc = bacc.Bacc(
        "TRN2",
        target_bir_lowering=False,
        debug=False,
        enable_asserts=False,
        num_devices=NCORES,
    )
    tokT = nc.dram_tensor("tokT", [D, N], BF16, kind="ExternalInput")
    lab_bc = nc.dram_tensor("lab_bc", [128, N], BF16, kind="ExternalInput")
    lab_rows = nc.dram_tensor("lab_rows", [128, NB], FP32, kind="ExternalInput")
    out_p = nc.dram_tensor("part", [128, 4 * NB], FP32, kind="ExternalOutput")
    out_cs = nc.dram_tensor("cs", [1, NQ * 512], BF16, kind="ExternalOutput")

    A = 2.0 / (tau * tau)  # (dist/tau)^2 = A - A*sim

    act_chain = []  # ACT instructions in required execution order

    def act(*args, **kwargs):
        inst = nc.scalar.activation(*args, **kwargs)
        act_chain.append(inst)
        return inst

    with tile.TileContext(nc) as tc:
        with (
            tc.tile_pool(name="persist", bufs=1) as pp,
            tc.tile_pool(name="traw", bufs=8) as trp,
            tc.tile_pool(name="dist", bufs=5) as dsp,
            tc.tile_pool(name="sq", bufs=4) as sqp,
            tc.tile_pool(name="msk", bufs=5) as mkp,
            tc.tile_pool(name="psim", bufs=3, space="PSUM") as psim,
            tc.tile_pool(name="psm", bufs=2, space="PSUM") as psm,
        ):
            # ---- persistent tiles ----
            t8 = pp.tile([128, KT, W], FP8, tag="t8")
            Lc = pp.tile([128, W], BF16, tag="Lc")
            lr = pp.tile([128, NB], FP32, tag="lr")
            inv_bc = pp.tile([128, W], BF16, tag="inv_bc")
            dm0 = pp.tile([128, 128], BF16, tag="dm0")
            ones = pp.tile([128, 128], BF16, tag="ones")
            colacc = pp.tile([128, NQ * 512], BF16, tag="colacc")
            # partials: [rowsum | numpart | sdiag | s32]
            parts = pp.tile([128, 4 * NB], FP32, tag="parts")
            biasA = pp.tile([128, 1], FP32, tag="biasA")
            biasE = pp.tile([128, 1], FP32, tag="biasE")

            nc.gpsimd.memset(ones[:], 1.0)
            nc.gpsimd.memset(biasA[:], float(A))
            nc.gpsimd.memset(biasE[:], float(ESC * np.log(2.0)))
            nc.gpsimd.memset(colacc[:], 0.0)

            # ---- diag mask dm0[p, f] = (f == p) ----
            iot = mkp.tile([128, 128], mybir.dt.int32, tag="iot")
            nc.gpsimd.iota(iot[:], pattern=[[1, 128]], base=0, channel_multiplier=-1)
            iotf = mkp.tile([128, 128], FP32, tag="iotf")
            nc.vector.tensor_copy(iotf[:], iot[:])
            nc.vector.tensor_scalar(dm0[:], iotf[:], 0.0, None, op0=OP.is_equal)

            # ---- prep helper: load tokT group, norms, normalize -> fp8 ----
            # only local cols [0, W) are ever used by the band scheme
            def prep_group(g):
                gsl = slice(g * GW2, (g + 1) * GW2)
                tch = []
                for k in range(KT):
                    t = trp.tile([128, GW2], BF16, tag="tr", name=f"tr{g}_{k}")
                    tch.append(t)
                    nc.sync.dma_start(
                        t[:], tokT[k * 128:(k + 1) * 128, gsl],
                    )
                sqt = []
                for k in range(KT):
                    s = sqp.tile([128, GW2], BF16, tag="sq")
                    sqt.append(s)
                    nc.vector.tensor_tensor(s[:], tch[k][:], tch[k][:],
                                            op=OP.mult)
                for c in range(GW2 // 512):
                    csl_g = slice(c * 512, (c + 1) * 512)
                    csl = slice(g * GW2 + c * 512, g * GW2 + (c + 1) * 512)
                    nps = psm.tile([128, 512], FP32, tag="sm", name=f"nps{g}_{c}")
                    for k in range(KT):
                        nc.tensor.matmul(
                            nps[:, :], ones[:, :], sqt[k][:, csl_g],
                            start=(k == 0), stop=(k == KT - 1),
                        )
                    act(inv_bc[:, csl], nps[:], AF.Abs_reciprocal_sqrt)
                # normalize + cast to fp8 DoubleRow layout
                for k in range(KT):
                    nc.vector.tensor_tensor(
                        t8[:, k, gsl], tch[k][:], inv_bc[:, gsl], op=OP.mult,
                    )

            def sim_chunk(dist_m, m, ch):
                base = m * 128
                ps = psim.tile([128, CH], FP32, tag="ps")
                c0 = base + 128 + ch * CH
                for kp in range(KT // 2):
                    lhsT = t8[:, 2 * kp:2 * kp + 2, base:base + 128]
                    for nn in range(CH // 512):
                        cs = slice(c0 + nn * 512, c0 + (nn + 1) * 512)
                        nc.tensor.matmul(
                            ps[:, nn * 512:(nn + 1) * 512],
                            lhsT, t8[:, 2 * kp:2 * kp + 2, cs],
                            start=(kp == 0), stop=(kp == KT // 2 - 1),
                            perf_mode=DR,
                        )
                act(dist_m[:, 128 + ch * CH:128 + (ch + 1) * CH],
                    ps[:], AF.Sqrt, bias=biasA[:], scale=float(-A))

            def sim_diag(dist_m, m):
                base = m * 128
                dps = psm.tile([128, 512], FP32, tag="sm", name=f"dg{m}")
                for kp in range(KT // 2):
                    nc.tensor.matmul(
                        dps[:, 0:128],
                        t8[:, 2 * kp:2 * kp + 2, base:base + 128],
                        t8[:, 2 * kp:2 * kp + 2, base:base + 128],
                        start=(kp == 0), stop=(kp == KT // 2 - 1),
                        perf_mode=DR,
                    )
                nc.vector.scalar_tensor_tensor(
                    out=dps[:, 0:128], in0=dm0[:], scalar=-2.0,
                    in1=dps[:, 0:128], op0=OP.mult, op1=OP.add,
                )
                act(dist_m[:, 0:128], dps[:, 0:128], AF.Sqrt,
                    bias=biasA[:], scale=float(-A))

            def numerator(dist_m, m):
                # fused: (Lc == label_row) builds the positive-pair mask in
                # ALU stage 0, stage 1 multiplies by dist, accumulator
                # row-sums -- one DVE pass, no pos tile
                base = m * 128
                jnk = mkp.tile([128, BAND - 128], FP16, tag="jnk")
                for lo, hi, col in (
                    (128, BAND, NB + m),
                    (0, 128, 2 * NB + m),      # diag
                    (BAND, REG, 3 * NB + m),   # d=32
                ):
                    nc.vector.scalar_tensor_tensor(
                        out=jnk[:, 0:hi - lo], in0=Lc[:, base + lo:base + hi],
                        scalar=lr[:, m:m + 1], in1=dist_m[:, lo:hi],
                        op0=OP.is_equal, op1=OP.mult,
                        accum_out=parts[:, col:col + 1],
                    )

            # ---- main compute, prep interleaved with phase 0 ----
            def colsum(blocks, dist_of):
                for q in range(NQ):
                    q0, q1 = q * 512, (q + 1) * 512
                    pieces = []
                    for m in blocks:
                        lo = max(q0, m * 128 + 128)
                        hi = min(q1, m * 128 + 128 + CSW)
                        if lo < hi:
                            pieces.append((m, lo, hi))
                    if not pieces:
                        continue
                    pieces.sort(key=lambda t: t[1] - t[2])  # widest first
                    cps = psm.tile([128, 512], FP32, tag="sm",
                                   name=f"cs{blocks[0]}_{q}")
                    for i, (m, lo, hi) in enumerate(pieces):
                        nc.tensor.matmul(
                            cps[:, lo - q0:hi - q0],
                            ones[:, :],
                            dist_of[m][:, lo - m * 128:hi - m * 128],
                            start=(i == 0), stop=(i == len(pieces) - 1),
                        )
                    lo = min(p[1] for p in pieces)
                    hi = max(p[2] for p in pieces)
                    nc.vector.tensor_tensor(
                        colacc[:, lo:hi], colacc[:, lo:hi],
                        cps[:, lo - q0:hi - q0], op=OP.add,
                    )

            for g in range(3):
                prep_group(g)
            nc.sync.dma_start(Lc[:], lab_bc[:, 0:W])
            nc.sync.dma_start(lr[:], lab_rows[:, :])
            # phase 0 round A interleaved with remaining prep groups so the
            # ACT chain releases rsqrt(g3)/rsqrt(g4) early
            dist_of = {}
            for m in range(BPP):
                dist_of[m] = dsp.tile([128, REG], FP16, tag="ds",
                                      name=f"dist{m}")
                sim_diag(dist_of[m], m)
                sim_chunk(dist_of[m], m, 0)
            prep_group(3)
            for m in range(BPP):
                sim_chunk(dist_of[m], m, 1)
            prep_group(4)
            # phase 0 round B: remaining chunks, numerator, exp, colsum
            for m in range(BPP):
                sim_chunk(dist_of[m], m, 2)
                sim_chunk(dist_of[m], m, 3)
                numerator(dist_of[m], m)
            for m in range(BPP):
                act(dist_of[m][:], dist_of[m][:], AF.Exp, scale=-1.0,
                    bias=biasE[:], accum_out=parts[:, m:m + 1])
            colsum(list(range(BPP)), dist_of)
            # phase 1
            dist_of = {}
            for m in range(BPP, NB):
                dist_of[m] = dsp.tile([128, REG], FP16, tag="ds",
                                      name=f"dist{m}")
                sim_diag(dist_of[m], m)
                for ch in range(BAND // CH):
                    sim_chunk(dist_of[m], m, ch)
                numerator(dist_of[m], m)
            for m in range(BPP, NB):
                act(dist_of[m][:], dist_of[m][:], AF.Exp, scale=-1.0,
                    bias=biasE[:], accum_out=parts[:, m:m + 1])
                if m == NB - 3:
                    colsum([BPP, BPP + 1], dist_of)
            colsum([NB - 2, NB - 1], dist_of)

            nc.sync.dma_start(out_p[:, :], parts[:])
            nc.sync.dma_start(out_cs[:, :], colacc[0:1, :])

            # ---- pin ACT execution order (stop table-set thrash) ----
            for a, b in zip(act_chain, act_chain[1:]):
                add_dep_helper(b.ins, a.ins, reason="act table-set order")

    nc.compile()
    return nc


def _get_program(tau: float):
    if tau not in _CACHE:
        _CACHE[tau] = _build(tau)
    return _CACHE[tau]


def make_in_maps(tokens: np.ndarray, labels: np.ndarray):
    bf = ml_dtypes.bfloat16
    tokT_full = np.ascontiguousarray(
        np.asarray(tokens, dtype=np.float32).astype(bf).T)  # [D, N]
    lab = np.asarray(labels).astype(np.float32)
    in_maps = []
    for c in range(NCORES):
        sh = c * RPC
        tokT_rot = np.ascontiguousarray(np.roll(tokT_full, -sh, axis=1))
        lab_rot = np.roll(lab, -sh)
        lab_bc = np.ascontiguousarray(
            np.broadcast_to(lab_rot.astype(bf)[None, :], (128, N))
        )
        lab_rows = np.ascontiguousarray(
            lab_rot[:RPC].reshape(NB, 128).T.astype(np.float32)
        )
        in_maps.append({
            "tokT": tokT_rot,
            "lab_bc": lab_bc,
            "lab_rows": lab_rows,
        })
    return in_maps


def _install_ntff_hook_shim():
    """Provide antenv.axon_hooks if the image lacks it (NTFF profiling via
    direct ctypes calls into libaxon_pjrt.so)."""
    try:
        from antenv.axon_hooks import get_axon_ntff_profile_hook  # noqa: F401
        return True
    except ImportError:
        pass
    so_path = "/opt/axon/libaxon_pjrt.so"
    if not os.path.exists(so_path):
        return False
    import contextlib
    import ctypes
    import types

    lib = ctypes.CDLL(so_path)
    if not hasattr(lib, "axon_start_nrt_profile"):
        return False
    lib.axon_start_nrt_profile.argtypes = [
        ctypes.POINTER(ctypes.c_int64), ctypes.c_size_t,
    ]
    lib.axon_start_nrt_profile.restype = ctypes.c_int64
    lib.axon_stop_nrt_profile.argtypes = [ctypes.c_char_p]
    lib.axon_stop_nrt_profile.restype = ctypes.c_int64

    @contextlib.contextmanager
    def _hook(output_dir, device_ids):
        import jax
        jax.devices()
        if device_ids:
            ids = (ctypes.c_int64 * len(device_ids))(*device_ids)
            rc = lib.axon_start_nrt_profile(ids, len(device_ids))
        else:
            rc = lib.axon_start_nrt_profile(None, 0)
        if rc != 0:
            raise RuntimeError(f"axon_start_nrt_profile rc={rc}")
        try:
            yield
        finally:
            n = lib.axon_stop_nrt_profile(str(output_dir).encode())
            if n < 0:
                raise RuntimeError(f"axon_stop_nrt_profile rc={n}")
            print(f"profile: {n} file(s) written to {output_dir}")

    mod = types.ModuleType("antenv.axon_hooks")
    mod.get_axon_ntff_profile_hook = lambda: _hook
    mod.set_axon_ntff_profile_hook = lambda h: None
    sys.modules["antenv.axon_hooks"] = mod
    return True


def kernel(tokens, labels, temperature=0.07):
    global last_results
    tau = float(temperature)
    nc = _get_program(tau)
    lab = np.asarray(labels).astype(np.int64)
    in_maps = make_in_maps(tokens, lab)
    trace = bool(int(os.environ.get("KBENCH_TRACE", "0")))
    if trace:
        trace = _install_ntff_hook_shim()
    res = bass_utils.run_bass_kernel_spmd(
        nc, in_maps, core_ids=list(range(NCORES)),
        trace=trace,
    )
    last_results = res

    counts = np.bincount(lab, minlength=NCLS)
    npos = (counts[lab] - 1).astype(np.float64)
    den = npos.sum()

    rowsum = np.zeros(N, dtype=np.float64)
    extra = np.zeros(N, dtype=np.float64)
    num = 0.0
    for c in range(NCORES):
        p = res.results[c]["part"].astype(np.float64)   # [128, 4*NB]
        cs = res.results[c]["cs"].astype(np.float64)    # [1, NQ*512]
        base = c * RPC
        for m in range(NB):
            gl = base + m * 128
            rowsum[gl:gl + 128] = p[:, m]
        num += 2.0 * p[:, NB:2 * NB].sum()      # mid (d=1..31) counted twice
        num += p[:, 2 * NB:3 * NB].sum()        # diag once
        num += p[:, 3 * NB:4 * NB].sum()        # d=32 once
        loc = np.zeros(N, dtype=np.float64)
        loc[:NQ * 512] = cs[0]
        extra += np.roll(loc, base)
    rowsum += extra
    num -= N * (2.0 / tau)                 # self-pair correction
    lse = np.log(rowsum) - ESC * np.log(2.0)
    num += (npos * lse).sum()
    return np.float32(num / den)
